# revision 1
# baseline (speedup 1.0000x reference)
"""Criss-cross attention (CC module) Trainium2 Bass kernel, v3.

Shapes (full): x2,x1 [8, 512, 64, 64] fp32; q_w,k_w [64, 512]; v_w [512, 512];
biases; gamma [1]. Outputs (y2, y1) same shape as x2/x1.

Distribution: data-parallel over batch B=8 across the 8 NeuronCores, one batch
element per core. Host packs/unpacks; single-core SPMD NEFF.

Per-core algorithm (C=512, CQ=64, H=W=64, S=4096), all-bf16 matmuls:
  Phase B: psQ/psK [64,512] = qkw^T x2 (+bias K=1 MM) per 512-col block
    -> q_sb/k_sb [64, S].
  Phase C: energies once into block-diagonal att tiles (keys on partitions):
    att1 chunk j2 (cols 128j2..): quadrant (wp,wp) = exp(E_H)[H', h] for
      w=2j2+wp (H' plain order, h plain order).
    att2 chunk j (cols 128j..): quadrant (hp,hp) = exp(E_W)[W'em, w] for
      h=2j+hp; W' key rows in em order (wpar*32+u <-> W'=2u+wpar) to make
      the vt gather partition-clean.
  Phase D: Z' [128,512] = (1/g)*(colsum att1 w-major-contig + colsum att2
    via reorder AP) per s'-block; DVE reciprocal -> r_sb = g/Z (w-major).
  Phase E: att tiles *= r in place (zeros stay zero).
  Pass 1 (h-major chunks j): vt psum rows tau=(hp, wpar, u) em order via
    host-packed x2p/x1p chunk column order; out psum = ident-inject(xt
    chunk = x^T + g*v_b) + att2-block MM -> part strips (bf16, rows
    (hp, w) plain, cols (j c)). vt tiles gathered into w-major vtw strips
    (4 clean DMAs per chunk per tensor: 32-contig src rows -> 1 dst row).
  Boundary: part strips -> DRAM part2d/part1d in w-major row order
    (2 grouped DMAs per tensor).
  Pass 2 (w-major chunks j2, groups of 4): p2cg group loads from part2d;
    psF = att1-block MM x vtw block; y = DVE add(psF + p2cg) -> y strip
    -> grouped DMA to y2t/y1t rows s' = w*64+h. Host: y[c,h,w] = yt[s',c].
"""

import numpy as np
import ml_dtypes

import concourse.bass as bass
import concourse.mybir as mybir
import concourse.tile as tile
from concourse import bacc
from concourse.bass_utils import run_bass_kernel_spmd
from concourse.masks import make_identity

BF16 = mybir.dt.bfloat16
F32 = mybir.dt.float32

B, C, H, W = 8, 512, 64, 64
CQ = 64
S = H * W  # 4096
NCH = S // 128  # 32 spatial chunks of 128 rows
KC = C // 128  # 4 contraction chunks
NB = S // 512  # 8 column blocks of 512
NG = NCH // 4  # 8 groups of 4 chunks

_CACHED = {}


def build_nc(gamma: float):
    g = float(gamma)
    assert abs(g) > 1e-12, "gamma ~ 0 not supported by 1/g folding"
    ginv = 1.0 / g

    nc = bacc.Bacc("TRN2", target_bir_lowering=False, debug=False)

    x2h = nc.dram_tensor("x2h", [KC, 128, S], BF16, kind="ExternalInput")
    x2p = nc.dram_tensor("x2p", [NCH, 128, 512], BF16, kind="ExternalInput")
    x1p = nc.dram_tensor("x1p", [NCH, 128, 512], BF16, kind="ExternalInput")
    xt2g = nc.dram_tensor("xt2g", [S, C], BF16, kind="ExternalInput")
    xt1g = nc.dram_tensor("xt1g", [S, C], BF16, kind="ExternalInput")
    qkw = nc.dram_tensor("qkw", [KC, 128, 128], BF16, kind="ExternalInput")
    qkb = nc.dram_tensor("qkb", [1, 128], BF16, kind="ExternalInput")
    vwtb = nc.dram_tensor("vwtb", [KC, 128, C], BF16, kind="ExternalInput")

    y2t = nc.dram_tensor("y2t", [S, C], BF16, kind="ExternalOutput")
    y1t = nc.dram_tensor("y1t", [S, C], BF16, kind="ExternalOutput")

    # internal DRAM scratch: pass-1 partials in w-major row order s'=w*64+h
    part2d = nc.dram_tensor("part2d", [S, C], BF16)
    part1d = nc.dram_tensor("part1d", [S, C], BF16)

    # partition-outer DRAM views for grouped row-chunk transfers
    x2h_v = x2h.rearrange("k p s -> p k s")
    x2p_v = x2p.rearrange("j p c -> p j c")
    x1p_v = x1p.rearrange("j p c -> p j c")
    xt2g_v = xt2g.rearrange("(gr q p) c -> gr p q c", p=128, q=2)
    xt1g_v = xt1g.rearrange("(gr q p) c -> gr p q c", p=128, q=2)
    p2d_r = part2d.rearrange("(gr q p) c -> gr p q c", p=128, q=4)
    p1d_r = part1d.rearrange("(gr q p) c -> gr p q c", p=128, q=4)
    y2t_v = y2t.rearrange("(gr q p) c -> gr p q c", p=128, q=4)
    y1t_v = y1t.rearrange("(gr q p) c -> gr p q c", p=128, q=4)
    # w-major write views: row s' = w*64 + 2j + hp <- part strip[hp*64+w, j*512+c]
    p2d_w = part2d.rearrange("(w j hp) c -> hp w j c", j=32, hp=2)
    p1d_w = part1d.rearrange("(w j hp) c -> hp w j c", j=32, hp=2)

    with tile.TileContext(nc) as tc:
        with (
            tc.tile_pool(name="persist", bufs=1) as pp,
            tc.tile_pool(name="psum", bufs=8, space="PSUM") as psp,
        ):
            # ---- persistent tiles ----
            qkw_t = [pp.tile([128, 128], BF16, tag=f"qkw_{i}", name=f"qkw_{i}") for i in range(KC)]
            vwtb_t = [pp.tile([128, C], BF16, tag=f"vwtb_{i}", name=f"vwtb_{i}") for i in range(KC)]
            qkb_t = pp.tile([1, 128], BF16, tag="qkb", name="qkb")
            ones_row = pp.tile([1, 512], BF16, tag="ones_row", name="ones_row")
            ginv_t = pp.tile([128, 128], BF16, tag="ginv", name="ginv")
            ident = pp.tile([128, 128], BF16, tag="ident", name="ident")
            att1 = pp.tile([128, S], BF16, tag="att1", name="att1")
            att2 = pp.tile([128, S], BF16, tag="att2", name="att2")
            part2 = pp.tile([128, NCH * 512], BF16, tag="part2", name="part2")
            part1 = pp.tile([128, NCH * 512], BF16, tag="part1", name="part1")
            vtw2 = pp.tile([128, NCH * 512], BF16, tag="vtw2", name="vtw2")
            vtw1 = pp.tile([128, NCH * 512], BF16, tag="vtw1", name="vtw1")

            nc.gpsimd.memset(ones_row[:], 1.0)
            nc.gpsimd.memset(ginv_t[:], ginv)
            nc.vector.memset(att1[:], 0.0)
            nc.vector.memset(att2[:], 0.0)
            make_identity(nc, ident[:])

            nc.sync.dma_start(qkb_t[:], qkb[:])
            for i in range(KC):
                nc.sync.dma_start(qkw_t[i][:], qkw[i, :, :])
                nc.sync.dma_start(vwtb_t[i][:], vwtb[i, :, :])

            # ---- Phase B: q,k projection (scoped tiles, freed after E) ----
            ringB = tc.alloc_tile_pool(name="ringB", bufs=2)
            qkpool = tc.alloc_tile_pool(name="qkpool", bufs=1)
            q_sb = qkpool.tile([64, S], BF16, tag="q_sb", name="q_sb")
            k_sb = qkpool.tile([64, S], BF16, tag="k_sb", name="k_sb")
            r_sb = qkpool.tile([128, S], BF16, tag="r_sb", name="r_sb")
            for n in range(NB):
                cols = slice(512 * n, 512 * (n + 1))
                x2c = ringB.tile([128, KC * 512], BF16, tag="xc", bufs=2, name="x2c")
                nc.sync.dma_start(x2c[:], x2h_v[:, :, cols])
                psQ = psp.tile([64, 512], F32, tag="ps", name="ps_q")
                psK = psp.tile([64, 512], F32, tag="ps", name="ps_k")
                for kc in range(KC):
                    nc.tensor.matmul(
                        psQ[:],
                        lhsT=qkw_t[kc][:, 0:64],
                        rhs=x2c[:, 512 * kc : 512 * (kc + 1)],
                        start=(kc == 0),
                        stop=False,
                    )
                    nc.tensor.matmul(
                        psK[:],
                        lhsT=qkw_t[kc][:, 64:128],
                        rhs=x2c[:, 512 * kc : 512 * (kc + 1)],
                        start=(kc == 0),
                        stop=False,
                    )
                nc.tensor.matmul(
                    psQ[:], lhsT=qkb_t[0:1, 0:64], rhs=ones_row[:],
                    start=False, stop=True,
                )
                nc.tensor.matmul(
                    psK[:], lhsT=qkb_t[0:1, 64:128], rhs=ones_row[:],
                    start=False, stop=True,
                )
                nc.scalar.activation(
                    out=q_sb[:, cols], in_=psQ[:],
                    func=mybir.ActivationFunctionType.Copy,
                )
                # store k with em-permuted w: col = hh*64 + (w%2)*32 + w//2
                nc.scalar.activation(
                    out=k_sb[:, cols].rearrange(
                        "p (hh q u) -> p hh u q", q=2, u=32
                    ),
                    in_=psK[:].rearrange("p (hh u q) -> p hh u q", u=32, q=2),
                    func=mybir.ActivationFunctionType.Copy,
                )

            q_hw = q_sb[:].rearrange("p (h w) -> p h w", w=W)
            # k_sb cols are (h, em(w)); em(w) = (w%2)*32 + w//2
            k_emx = k_sb[:].rearrange("p (h x) -> p x h", x=64)

            # ---- Phase C: energies -> block-diagonal exp tiles ----
            for grp in range(8):
                psE1 = psp.tile([128, 512], F32, tag="ps", name="ps_e1")
                psE2 = psp.tile([128, 512], F32, tag="ps", name="ps_e2")
                for mm in range(8):
                    m = 8 * grp + mm
                    par = m % 2
                    rows = slice(64 * par, 64 * (par + 1))
                    cols = slice(128 * (mm // 2) + 64 * par,
                                 128 * (mm // 2) + 64 * (par + 1))
                    # E_H for column w=m: out [H' plain, h plain]
                    nc.tensor.matmul(
                        psE1[rows, cols],
                        lhsT=k_emx[:, (m % 2) * 32 + m // 2, :],
                        rhs=q_hw[:, :, m],
                        start=True, stop=True, skip_group_check=True,
                    )
                    # E_W for row h=m: out [W' em order, w plain]
                    nc.tensor.matmul(
                        psE2[rows, cols],
                        lhsT=k_sb[:, 64 * m : 64 * (m + 1)],
                        rhs=q_hw[:, m, :],
                        start=True, stop=True, skip_group_check=True,
                    )
                for par in range(2):
                    rows = slice(64 * par, 64 * (par + 1))
                    for ps_t, att in ((psE1, att1), (psE2, att2)):
                        src = ps_t[rows].rearrange(
                            "p (b q c) -> p b q c", b=4, q=2
                        )[:, :, par, :]
                        dst = att[rows, 512 * grp : 512 * (grp + 1)].rearrange(
                            "p (b q c) -> p b q c", b=4, q=2
                        )[:, :, par, :]
                        nc.scalar.activation(
                            out=dst, in_=src,
                            func=mybir.ActivationFunctionType.Exp,
                        )

            # ---- Phase D: Z' = (1/g)*(colsums), w-major query order ----
            att2_wmaj = att2[:].rearrange("p (u q w) -> p w u q", u=32, q=2, w=64)
            for n in range(NB):
                cols = slice(512 * n, 512 * (n + 1))
                psZ = psp.tile([128, 512], F32, tag="ps", name="ps_z")
                nc.tensor.matmul(
                    psZ[:], lhsT=ginv_t[:], rhs=att1[:, cols],
                    start=True, stop=False,
                )
                nc.tensor.matmul(
                    psZ[:], lhsT=ginv_t[:],
                    rhs=att2_wmaj[:, 8 * n : 8 * (n + 1), :, :],
                    start=False, stop=True,
                )
                with nc.allow_low_precision(reason="softmax recip row in bf16"):
                    nc.vector.reciprocal(r_sb[:, cols], psZ[:])

            # ---- Phase E: normalize att tiles in place ----
            r_hmaj = r_sb[:].rearrange("p (w u q) -> p u q w", w=64, u=32, q=2)
            for n in range(NB):
                cols = slice(512 * n, 512 * (n + 1))
                nc.vector.tensor_mul(att1[:, cols], att1[:, cols], r_sb[:, cols])
                nc.vector.tensor_mul(
                    att2[:, cols], att2[:, cols],
                    r_hmaj[:, 4 * n : 4 * (n + 1), :, :],
                )

            qkpool.release()
            ringB.release()
            ringP = tc.alloc_tile_pool(name="ringP", bufs=2)


            # ---- Pass 1: W-attention over h-major chunks ----
            for j in range(NCH):
                gi, jj = j // 4, j % 4
                gi2, jj2 = j // 2, j % 2
                if jj == 0:
                    x2cc = ringP.tile([128, 4 * 512], BF16, tag="x2cc", bufs=2, name="x2cc")
                    nc.sync.dma_start(x2cc[:], x2p_v[:, 4 * gi : 4 * gi + 4, :])
                    x1cc = ringP.tile([128, 4 * 512], BF16, tag="x1cc", bufs=2, name="x1cc")
                    nc.sync.dma_start(x1cc[:], x1p_v[:, 4 * gi : 4 * gi + 4, :])
                if jj2 == 0:
                    xt2c = ringP.tile([128, 2 * 512], BF16, tag="xt2c", bufs=2, name="xt2c")
                    nc.sync.dma_start(xt2c[:], xt2g_v[gi2])
                    xt1c = ringP.tile([128, 2 * 512], BF16, tag="xt1c", bufs=2, name="xt1c")
                    nc.sync.dma_start(xt1c[:], xt1g_v[gi2])

                attb = att2[:, 128 * j : 128 * (j + 1)]
                for ti, (xcc, xtc, vtw_t, part) in enumerate((
                    (x2cc, xt2c, vtw2, part2),
                    (x1cc, xt1c, vtw1, part1),
                )):
                    sfx = "2" if ti == 0 else "1"
                    psV = psp.tile([128, C], F32, tag="ps", name=f"ps_v{sfx}")
                    for kc in range(KC):
                        nc.tensor.matmul(
                            psV[:],
                            lhsT=xcc[:, 512 * jj + 128 * kc : 512 * jj + 128 * (kc + 1)],
                            rhs=vwtb_t[kc][:],
                            start=(kc == 0),
                            stop=(kc == KC - 1),
                        )
                    # vt rows in tau order: r = hp*64 + wpar*32 + u
                    vt = ringP.tile([128, C], BF16, tag=f"vt{sfx}", bufs=2, name=f"vt{sfx}")
                    if ti == 0:
                        nc.scalar.activation(
                            out=vt[:], in_=psV[:],
                            func=mybir.ActivationFunctionType.Copy,
                        )
                    else:
                        nc.vector.tensor_copy(vt[:], psV[:])
                    # gather: vtw[wq*64 + 2j + hp, u*512 + c] = vt[hp*64+wq*32+u, c]
                    for hp in range(2):
                        for wq in range(2):
                            eng = nc.gpsimd if (hp == 0) else nc.sync
                            p0 = wq * 64 + 2 * j + hp
                            eng.dma_start(
                                vtw_t[p0 : p0 + 1, :],
                                vt[64 * hp + 32 * wq : 64 * hp + 32 * (wq + 1), :],
                            )
                    psO = psp.tile([128, C], F32, tag="ps", name=f"ps_o{sfx}")
                    nc.tensor.matmul(
                        psO[:], lhsT=ident[:],
                        rhs=xtc[:, 512 * jj2 : 512 * (jj2 + 1)],
                        start=True, stop=False, skip_group_check=True,
                    )
                    nc.tensor.matmul(
                        psO[:], lhsT=attb, rhs=vt[:],
                        start=False, stop=True, skip_group_check=True,
                    )
                    if ti == 0:
                        nc.vector.tensor_copy(
                            part[:, 512 * j : 512 * (j + 1)], psO[:]
                        )
                    else:
                        nc.scalar.activation(
                            out=part[:, 512 * j : 512 * (j + 1)], in_=psO[:],
                            func=mybir.ActivationFunctionType.Copy,
                        )

            # ---- boundary: part strips -> DRAM in w-major row order ----
            p2s = part2[:].rearrange("(hp w) (j c) -> hp w j c", hp=2, c=512)
            p1s = part1[:].rearrange("(hp w) (j c) -> hp w j c", hp=2, c=512)
            for hp in range(2):
                nc.sync.dma_start(p2d_w[hp], part2[64 * hp : 64 * (hp + 1), :].rearrange("w (j c) -> w j c", c=512))
                nc.sync.dma_start(p1d_w[hp], part1[64 * hp : 64 * (hp + 1), :].rearrange("w (j c) -> w j c", c=512))

            # ---- Pass 2: H-attention over w-major chunks, groups of 4 ----
            for gi in range(NG):
                p2cg = ringP.tile([128, 4 * 512], BF16, tag="p2cg", bufs=2, name="p2cg")
                nc.sync.dma_start(p2cg[:], p2d_r[gi])
                p1cg = ringP.tile([128, 4 * 512], BF16, tag="p1cg", bufs=2, name="p1cg")
                nc.sync.dma_start(p1cg[:], p1d_r[gi])
                ys2 = ringP.tile([128, 4 * 512], BF16, tag="ys2", bufs=1, name="ys2")
                ys1 = ringP.tile([128, 4 * 512], BF16, tag="ys1", bufs=1, name="ys1")
                for q in range(4):
                    j2 = 4 * gi + q
                    attb = att1[:, 128 * j2 : 128 * (j2 + 1)]
                    for ti, (pcg, vtw, ys) in enumerate((
                        (p2cg, vtw2, ys2), (p1cg, vtw1, ys1),
                    )):
                        psF = psp.tile([128, C], F32, tag="ps", name=f"ps_f{ti}")
                        nc.tensor.matmul(
                            psF[:], lhsT=attb,
                            rhs=vtw[:, 512 * j2 : 512 * (j2 + 1)],
                            start=True, stop=True,
                        )
                        nc.vector.tensor_add(
                            ys[:, 512 * q : 512 * (q + 1)],
                            psF[:],
                            pcg[:, 512 * q : 512 * (q + 1)],
                        )
                nc.gpsimd.dma_start(y2t_v[gi], ys2[:])
                nc.gpsimd.dma_start(y1t_v[gi], ys1[:])

            ringP.release()

    nc.compile()
    return nc


def make_in_maps(x2, x1, q_w, q_b, k_w, k_b, v_w, v_b, gamma):
    x2 = np.asarray(x2, dtype=np.float32)
    x1 = np.asarray(x1, dtype=np.float32)
    g = float(np.asarray(gamma).reshape(-1)[0])
    bf16 = ml_dtypes.bfloat16
    qkw = (
        np.concatenate([np.asarray(q_w).T, np.asarray(k_w).T], axis=1)
        .reshape(KC, 128, 128)
        .astype(bf16)
    )
    qkb = np.concatenate([np.asarray(q_b), np.asarray(k_b)]).reshape(1, 128).astype(bf16)
    vwtb = np.asarray(v_w).T.reshape(KC, 128, C).astype(bf16)
    gbv = (g * np.asarray(v_b)).astype(np.float32)  # [C]

    def pack_p(xfl):
        # [C, S] -> [NCH, 128, 512]; chunk cols in tau order:
        # col r = hp*64 + wpar*32 + u  <->  s = (2j+hp)*64 + 2u + wpar
        t = xfl.reshape(KC, 128, NCH, 2, 32, 2)  # kc p j hp u wpar
        return np.ascontiguousarray(
            t.transpose(2, 1, 0, 3, 5, 4).reshape(NCH, 128, KC * 128).astype(bf16)
        )

    in_maps = []
    for b in range(B):
        x2fl = x2[b].reshape(C, S)
        x1fl = x1[b].reshape(C, S)
        in_maps.append(
            {
                "x2h": np.ascontiguousarray(x2fl.reshape(KC, 128, S).astype(bf16)),
                "x2p": pack_p(x2fl),
                "x1p": pack_p(x1fl),
                "xt2g": np.ascontiguousarray((x2fl.T + gbv[None, :]).astype(bf16)),
                "xt1g": np.ascontiguousarray((x1fl.T + gbv[None, :]).astype(bf16)),
                "qkw": qkw,
                "qkb": qkb,
                "vwtb": vwtb,
            }
        )

    return in_maps, g


def assemble_outputs(res):
    y2 = np.empty((B, C, H, W), np.float32)
    y1 = np.empty((B, C, H, W), np.float32)
    for b in range(B):
        y2[b] = unpermute(np.asarray(res[b]["y2t"]))
        y1[b] = unpermute(np.asarray(res[b]["y1t"]))
    return y2, y1


def unpermute(yt):
    # yt row s' = w*64 + h -> y[c, h, w]
    return np.ascontiguousarray(
        yt.astype(np.float32).reshape(W, H, C).transpose(2, 1, 0)
    )


def kernel(x2, x1, q_w, q_b, k_w, k_b, v_w, v_b, gamma):
    in_maps, g = make_in_maps(x2, x1, q_w, q_b, k_w, k_b, v_w, v_b, gamma)
    key = round(g, 12)
    if key not in _CACHED:
        _CACHED[key] = build_nc(g)
    nc = _CACHED[key]
    res = run_bass_kernel_spmd(nc, in_maps, list(range(B))).results
    return assemble_outputs(res)



# revision 5
# speedup vs baseline: 1.1880x; 1.1880x over previous
"""Criss-cross attention (CC module) Trainium2 Bass kernel, v4 (c-major).

Shapes (full): x2,x1 [8, 512, 64, 64] fp32; q_w,k_w [64, 512]; v_w [512, 512];
biases; gamma [1]. Outputs (y2, y1) same shape as x2/x1.

Distribution: data-parallel over batch B=8 across the 8 NeuronCores.

v4 design: all attention outputs computed CHANNEL-major ([C, S]) by using v
as the stationary matmul operand, so the h-major <-> w-major partial regroup
is a pure column access pattern (no partition crossing, no DRAM round trip,
no SBUF scatter). x is host-packed in both h-major (x?p) and w-major (x?pw)
chunk orders; pass 2 recomputes V from x?pw instead of gathering. gamma is
folded into vwtb host-side; the residual x and the g*v_b bias (softmax rows
sum to 1) are added on the host.

Index maps (per chunk j of 128 spatial positions):
  h-major chunk j: col = kc*128 + hp*64 + em, em=(w%2)*32+w//2, h=2j+hp
  w-major chunk j2: col = kc*128 + wq*64 + h, w=2*j2+wq
  att1 chunk j2 quadrant (wp,wp): [h' plain, h plain] for w=2*j2+wp
  att2 chunk j  quadrant (hp,hp): [W' em, w em] for h=2j+hp
  partT col = cb*4096 + j*128 + hp*64 + em ; y2t col s'' = j2*128 + wp*64 + h
"""

import numpy as np
import ml_dtypes

import concourse.bass as bass
import concourse.mybir as mybir
import concourse.tile as tile
from concourse import bacc
from concourse.bass_utils import run_bass_kernel_spmd

BF16 = mybir.dt.bfloat16
F32 = mybir.dt.float32

B, C, H, W = 8, 512, 64, 64
CQ = 64
S = H * W  # 4096
NCH = S // 128  # 32 spatial chunks of 128
KC = C // 128  # 4 contraction chunks

_CACHED = {}


def build_nc():
    nc = bacc.Bacc("TRN2", target_bir_lowering=False, debug=False)

    x2p = nc.dram_tensor("x2p", [NCH, 128, 512], BF16, kind="ExternalInput")
    x1p = nc.dram_tensor("x1p", [NCH, 128, 512], BF16, kind="ExternalInput")
    x2pw = nc.dram_tensor("x2pw", [NCH, 128, 512], BF16, kind="ExternalInput")
    x1pw = nc.dram_tensor("x1pw", [NCH, 128, 512], BF16, kind="ExternalInput")
    qkw = nc.dram_tensor("qkw", [KC, 128, 128], BF16, kind="ExternalInput")
    qkb2 = nc.dram_tensor("qkb2", [64, 2], BF16, kind="ExternalInput")
    vwtb = nc.dram_tensor("vwtb", [KC, 128, C], BF16, kind="ExternalInput")

    y2t = nc.dram_tensor("y2t", [C, S], BF16, kind="ExternalOutput")
    y1t = nc.dram_tensor("y1t", [C, S], BF16, kind="ExternalOutput")

    x2p_v = x2p.rearrange("j p c -> p j c")
    x1p_v = x1p.rearrange("j p c -> p j c")
    x2pw_v = x2pw.rearrange("j p c -> p j c")
    x1pw_v = x1pw.rearrange("j p c -> p j c")
    # output views: y[(cb p), (gi sg)] <- ys group tile [p, (cb sg)]
    y2t_v = y2t.rearrange("(cb p) (gi sg) -> gi p cb sg", p=128, sg=512)
    y1t_v = y1t.rearrange("(cb p) (gi sg) -> gi p cb sg", p=128, sg=512)

    with tile.TileContext(nc) as tc:
        with (
            tc.tile_pool(name="persist", bufs=1) as pp,
            tc.tile_pool(name="psA", bufs=6, space="PSUM") as psA,
            tc.tile_pool(name="psB", bufs=2, space="PSUM") as psB,
        ):
            # ---- persistent tiles ----
            qkw_t = [pp.tile([128, 128], BF16, tag=f"qkw_{i}", name=f"qkw_{i}") for i in range(KC)]
            vwtb_t = [pp.tile([128, C], BF16, tag=f"vwtb_{i}", name=f"vwtb_{i}") for i in range(KC)]
            qkb_t = pp.tile([64, 2], BF16, tag="qkb", name="qkb")
            ones_col = pp.tile([128, 1], BF16, tag="ones_col", name="ones_col")
            ones1 = pp.tile([1, 128], BF16, tag="ones1", name="ones1")
            att1 = pp.tile([128, S], BF16, tag="att1", name="att1")
            att2 = pp.tile([128, S], BF16, tag="att2", name="att2")
            vth2 = pp.tile([128, NCH * 512], BF16, tag="vth2", name="vth2")
            partT2 = pp.tile([128, 4 * S], BF16, tag="partT2", name="partT2")
            partT1 = pp.tile([128, 4 * S], BF16, tag="partT1", name="partT1")
            r_sb = pp.tile([1, S], BF16, tag="r_sb", name="r_sb")
            r2_sb = pp.tile([1, S], BF16, tag="r2_sb", name="r2_sb")

            nc.gpsimd.memset(ones_col[:], 1.0)
            nc.gpsimd.memset(ones1[:], 1.0)
            nc.vector.memset(att1[:], 0.0)
            nc.vector.memset(att2[:], 0.0)

            nc.sync.dma_start(qkb_t[:], qkb2[:])
            for i in range(KC):
                nc.sync.dma_start(qkw_t[i][:], qkw[i, :, :])
                nc.sync.dma_start(vwtb_t[i][:], vwtb[i, :, :])

            qkpool = tc.alloc_tile_pool(name="qkpool", bufs=1)
            q_sb = qkpool.tile([64, S], BF16, tag="q_sb", name="q_sb")
            k_sb = qkpool.tile([64, S], BF16, tag="k_sb", name="k_sb")
            ringA = tc.alloc_tile_pool(name="ringA", bufs=2)

            # ---- proj pass over x2: Q/K, E_W, V2 ----
            psE2 = None
            for j in range(NCH):
                gi, jj = j // 4, j % 4
                if jj == 0:
                    x2c = ringA.tile([128, 4 * 512], BF16, tag="x2c", bufs=2, name="x2c")
                    nc.sync.dma_start(x2c[:], x2p_v[:, 4 * gi: 4 * gi + 4, :])
                xc = x2c[:, 512 * jj: 512 * (jj + 1)]
                psQK = psB.tile([64, 256], F32, tag="psqk", name="psqk")
                for kc in range(KC):
                    nc.tensor.matmul(
                        psQK[:, 0:128],
                        lhsT=qkw_t[kc][:, 0:64],
                        rhs=xc[:, 128 * kc: 128 * (kc + 1)],
                        start=(kc == 0), stop=(kc == KC - 1),
                    )
                for kc in range(KC):
                    nc.tensor.matmul(
                        psQK[:, 128:256],
                        lhsT=qkw_t[kc][:, 64:128],
                        rhs=xc[:, 128 * kc: 128 * (kc + 1)],
                        start=(kc == 0), stop=(kc == KC - 1),
                    )
                nc.scalar.activation(
                    out=q_sb[:, 128 * j: 128 * (j + 1)], in_=psQK[:, 0:128],
                    func=mybir.ActivationFunctionType.Identity, bias=qkb_t[:, 0:1],
                )
                nc.scalar.activation(
                    out=k_sb[:, 128 * j: 128 * (j + 1)], in_=psQK[:, 128:256],
                    func=mybir.ActivationFunctionType.Identity, bias=qkb_t[:, 1:2],
                )
                # E_W for h=2j+hp (keys em, queries em)
                if jj == 0:
                    psE2 = psA.tile([128, 512], F32, tag="ps", name="psE2")
                for hp in range(2):
                    sl = slice(128 * j + 64 * hp, 128 * j + 64 * hp + 64)
                    nc.tensor.matmul(
                        psE2[64 * hp: 64 * hp + 64,
                             128 * jj + 64 * hp: 128 * jj + 64 * hp + 64],
                        lhsT=k_sb[:, sl], rhs=q_sb[:, sl],
                        start=True, stop=True, skip_group_check=True,
                        tile_position=(0, 64 * hp),
                    )
                # V2 projection -> vth2
                psV = psA.tile([128, 512], F32, tag="ps", name="psV2")
                for kc in range(KC):
                    nc.tensor.matmul(
                        psV[:],
                        lhsT=xc[:, 128 * kc: 128 * (kc + 1)],
                        rhs=vwtb_t[kc][:],
                        start=(kc == 0), stop=(kc == KC - 1),
                    )
                nc.vector.tensor_copy(vth2[:, 512 * j: 512 * (j + 1)], psV[:])
                if jj == 3:
                    # exp psE2 -> att2 quadrants for this group of 4 chunks
                    att2_g = att2[:].rearrange("p (j hq) -> p j hq", hq=128)
                    psE2_g = psE2[:].rearrange("p (jl hq) -> p jl hq", hq=128)
                    for hp in range(2):
                        nc.scalar.activation(
                            out=att2_g[64 * hp: 64 * hp + 64, 4 * gi: 4 * gi + 4,
                                       64 * hp: 64 * hp + 64],
                            in_=psE2_g[64 * hp: 64 * hp + 64, :, 64 * hp: 64 * hp + 64],
                            func=mybir.ActivationFunctionType.Exp,
                        )

            # ---- E_H (64 mms) ----
            k_col = k_sb[:].rearrange("p (j hp em) -> p em (j hp)", hp=2, em=64)
            q_col = q_sb[:].rearrange("p (j hp em) -> p em (j hp)", hp=2, em=64)
            att1_g = att1[:].rearrange("p (j hq) -> p j hq", hq=128)
            for g8 in range(8):
                psE1 = psA.tile([128, 512], F32, tag="ps", name="psE1")
                psE1_g = psE1[:].rearrange("p (jl hq) -> p jl hq", hq=128)
                for wl in range(8):
                    w = 8 * g8 + wl
                    wp, j2l = w % 2, (w // 2) % 4
                    em = (w % 2) * 32 + w // 2
                    nc.tensor.matmul(
                        psE1[64 * wp: 64 * wp + 64,
                             128 * j2l + 64 * wp: 128 * j2l + 64 * wp + 64],
                        lhsT=k_col[:, em, :], rhs=q_col[:, em, :],
                        start=True, stop=True, skip_group_check=True,
                        tile_position=(0, 64 * wp),
                    )
                for wp in range(2):
                    nc.scalar.activation(
                        out=att1_g[64 * wp: 64 * wp + 64, 4 * g8: 4 * g8 + 4,
                                   64 * wp: 64 * wp + 64],
                        in_=psE1_g[64 * wp: 64 * wp + 64, :, 64 * wp: 64 * wp + 64],
                        func=mybir.ActivationFunctionType.Exp,
                    )

            # ---- colsum Z -> r (att1 col order), r2 (att2 col order) ----
            # att2 view: [p, j2, wp, (j hp)] with col = j*128+hp*64+wp*32+j2
            att2_zv = att2[:].rearrange(
                "p (j hp wp j2) -> p j2 wp (j hp)", hp=2, wp=2, j2=32
            )
            for n in range(8):
                psZ = psA.tile([128, 512], F32, tag="ps", name="psZ")
                nc.tensor.matmul(
                    psZ[0:1, :], lhsT=ones_col[:], rhs=att1[:, 512 * n: 512 * (n + 1)],
                    start=True, stop=False,
                )
                for j2l in range(4):
                    nc.tensor.matmul(
                        psZ[0:1, 128 * j2l: 128 * (j2l + 1)],
                        lhsT=ones_col[:], rhs=att2_zv[:, 4 * n + j2l, :, :],
                        start=False, stop=(j2l == 3),
                    )
                with nc.allow_low_precision(reason="softmax recip row in bf16"):
                    nc.vector.reciprocal(r_sb[:, 512 * n: 512 * (n + 1)], psZ[0:1, :])
            # r2[j*128+hp*64+wp*32+j2] = r[j2*128+wp*64+2j+hp]; per hp: 3 free dims
            r_src = r_sb[:].rearrange("o (j2 wp j hp) -> o hp j wp j2", j2=32, wp=2, hp=2)
            r2_dst = r2_sb[:].rearrange("o (j hp wp j2) -> o hp j wp j2", j=32, hp=2, wp=2)
            for hp in range(2):
                nc.vector.tensor_copy(r2_dst[:, hp], r_src[:, hp])
            # normalize att tiles via matmul row-broadcast of r
            for n in range(8):
                cols = slice(512 * n, 512 * (n + 1))
                psR = psA.tile([128, 512], F32, tag="ps", name="psR")
                nc.tensor.matmul(psR[:], lhsT=ones1[:], rhs=r_sb[:, cols],
                                 start=True, stop=True)
                nc.vector.tensor_mul(att1[:, cols], att1[:, cols], psR[:])
                psR2 = psA.tile([128, 512], F32, tag="ps", name="psR2")
                nc.tensor.matmul(psR2[:], lhsT=ones1[:], rhs=r2_sb[:, cols],
                                 start=True, stop=True)
                nc.vector.tensor_mul(att2[:, cols], att2[:, cols], psR2[:])

            ringA.release()
            qkpool.release()
            ringP = tc.alloc_tile_pool(name="ringP", bufs=2)

            # partT views: col = cb*4096 + scol
            pT2_v = partT2[:].rearrange("p (cb s) -> p cb s", cb=4)
            pT1_v = partT1[:].rearrange("p (cb s) -> p cb s", cb=4)

            # ---- pass 1: h-major chunks; V1 proj + c-major att_W partials ----
            for j in range(NCH):
                gi, jj = j // 4, j % 4
                if jj == 0:
                    x1c = ringP.tile([128, 4 * 512], BF16, tag="x1c", bufs=2, name="x1c")
                    nc.sync.dma_start(x1c[:], x1p_v[:, 4 * gi: 4 * gi + 4, :])
                xc = x1c[:, 512 * jj: 512 * (jj + 1)]
                psV = psA.tile([128, 512], F32, tag="ps", name="psV1")
                for kc in range(KC):
                    nc.tensor.matmul(
                        psV[:], lhsT=xc[:, 128 * kc: 128 * (kc + 1)],
                        rhs=vwtb_t[kc][:],
                        start=(kc == 0), stop=(kc == KC - 1),
                    )
                vt1 = ringP.tile([128, 512], BF16, tag="vt1", bufs=3, name="vt1")
                nc.scalar.activation(out=vt1[:], in_=psV[:],
                                     func=mybir.ActivationFunctionType.Copy)
                att2c = att2[:, 128 * j: 128 * (j + 1)]
                psO2 = psA.tile([128, 512], F32, tag="ps", name="psO2")
                for cb in range(4):
                    nc.tensor.matmul(
                        psO2[:, 128 * cb: 128 * (cb + 1)],
                        lhsT=vth2[:, 512 * j + 128 * cb: 512 * j + 128 * (cb + 1)],
                        rhs=att2c, start=True, stop=True, skip_group_check=True,
                    )
                nc.vector.tensor_copy(pT2_v[:, :, 128 * j: 128 * (j + 1)], psO2[:])
                psO1 = psA.tile([128, 512], F32, tag="ps", name="psO1")
                for cb in range(4):
                    nc.tensor.matmul(
                        psO1[:, 128 * cb: 128 * (cb + 1)],
                        lhsT=vt1[:, 128 * cb: 128 * (cb + 1)],
                        rhs=att2c, start=True, stop=True, skip_group_check=True,
                    )
                nc.scalar.activation(out=pT1_v[:, :, 128 * j: 128 * (j + 1)], in_=psO1[:],
                                     func=mybir.ActivationFunctionType.Copy)

            # ---- pass 2: w-major chunks; V recompute + att_H + combine ----
            # partT combine view: [p, cb, j2, (wp j hp)], col = cb*4096+j*128+hp*64+wp*32+j2
            pT2_c = partT2[:].rearrange(
                "p (cb j hp wp j2) -> p cb j2 wp (j hp)", cb=4, j=32, hp=2, wp=2
            )
            pT1_c = partT1[:].rearrange(
                "p (cb j hp wp j2) -> p cb j2 wp (j hp)", cb=4, j=32, hp=2, wp=2
            )
            for j2 in range(NCH):
                gi, jj = j2 // 4, j2 % 4
                if jj == 0:
                    x2wc = ringP.tile([128, 4 * 512], BF16, tag="x2wc", bufs=2, name="x2wc")
                    nc.sync.dma_start(x2wc[:], x2pw_v[:, 4 * gi: 4 * gi + 4, :])
                    x1wc = ringP.tile([128, 4 * 512], BF16, tag="x1wc", bufs=2, name="x1wc")
                    nc.sync.dma_start(x1wc[:], x1pw_v[:, 4 * gi: 4 * gi + 4, :])
                    ys2 = ringP.tile([128, 4 * 512], BF16, tag="ys2", bufs=2, name="ys2")
                    ys1 = ringP.tile([128, 4 * 512], BF16, tag="ys1", bufs=2, name="ys1")
                att1c = att1[:, 128 * j2: 128 * (j2 + 1)]
                for ti, (xwc, ysx, pT_c) in enumerate(
                    ((x2wc, ys2, pT2_c), (x1wc, ys1, pT1_c))
                ):
                    xcw = xwc[:, 512 * jj: 512 * (jj + 1)]
                    psVw = psA.tile([128, 512], F32, tag="ps", name=f"psVw{ti}")
                    for kc in range(KC):
                        nc.tensor.matmul(
                            psVw[:], lhsT=xcw[:, 128 * kc: 128 * (kc + 1)],
                            rhs=vwtb_t[kc][:],
                            start=(kc == 0), stop=(kc == KC - 1),
                        )
                    vtw = ringP.tile([128, 512], BF16, tag=f"vtw{ti}", bufs=3,
                                     name=f"vtw{ti}")
                    if ti == 0:
                        nc.scalar.activation(out=vtw[:], in_=psVw[:],
                                             func=mybir.ActivationFunctionType.Copy)
                    else:
                        nc.scalar.activation(out=vtw[:], in_=psVw[:],
                                             func=mybir.ActivationFunctionType.Copy)
                    psF = psA.tile([128, 512], F32, tag="ps", name=f"psF{ti}")
                    for cb in range(4):
                        nc.tensor.matmul(
                            psF[:, 128 * cb: 128 * (cb + 1)],
                            lhsT=vtw[:, 128 * cb: 128 * (cb + 1)],
                            rhs=att1c, start=True, stop=True, skip_group_check=True,
                        )
                    # combine: ys[:, cb*512 + jj*128 ..] = psF_cb + partT AP
                    ys_v = ysx[:].rearrange("p (cb jl q) -> p cb jl q", cb=4, q=128)
                    for cb in range(4):
                        nc.vector.tensor_add(
                            ys_v[:, cb, jj, :],
                            psF[:, 128 * cb: 128 * (cb + 1)],
                            pT_c[:, cb, j2, :, :],
                        )
                if jj == 3:
                    nc.gpsimd.dma_start(y2t_v[gi], ys2[:])
                    nc.gpsimd.dma_start(y1t_v[gi], ys1[:])

            ringP.release()

    nc.compile()
    return nc


def make_in_maps(x2, x1, q_w, q_b, k_w, k_b, v_w, v_b, gamma):
    x2 = np.asarray(x2, dtype=np.float32)
    x1 = np.asarray(x1, dtype=np.float32)
    g = float(np.asarray(gamma).reshape(-1)[0])
    bf16 = ml_dtypes.bfloat16
    qkw = (
        np.concatenate([np.asarray(q_w).T, np.asarray(k_w).T], axis=1)
        .reshape(KC, 128, 128).astype(bf16)
    )
    qkb2 = np.stack([np.asarray(q_b), np.asarray(k_b)], axis=1).astype(bf16)  # [64,2]
    vwtb = (g * np.asarray(v_w)).T.reshape(KC, 128, C).astype(bf16)

    def pack_p(xfl):
        t = xfl.reshape(KC, 128, NCH, 2, 32, 2)  # kc ch j hp u wpar
        return np.ascontiguousarray(
            t.transpose(2, 1, 0, 3, 5, 4).reshape(NCH, 128, KC * 128).astype(bf16)
        )

    def pack_pw(xfl):
        t = xfl.reshape(KC, 128, 64, 32, 2)  # kc ch h j2 wq
        return np.ascontiguousarray(
            t.transpose(3, 1, 0, 4, 2).reshape(NCH, 128, KC * 128).astype(bf16)
        )

    in_maps = []
    for b in range(B):
        x2fl = x2[b].reshape(C, S)
        x1fl = x1[b].reshape(C, S)
        in_maps.append(
            {
                "x2p": pack_p(x2fl),
                "x1p": pack_p(x1fl),
                "x2pw": pack_pw(x2fl),
                "x1pw": pack_pw(x1fl),
                "qkw": qkw,
                "qkb2": qkb2,
                "vwtb": vwtb,
            }
        )
    return in_maps, g


def assemble_outputs(res, x2, x1, v_b, g):
    y2 = np.empty((B, C, H, W), np.float32)
    y1 = np.empty((B, C, H, W), np.float32)
    gvb = (g * np.asarray(v_b, dtype=np.float32))[None, :, None, None]
    for b in range(B):
        y2[b] = unpermute(np.asarray(res[b]["y2t"]))
        y1[b] = unpermute(np.asarray(res[b]["y1t"]))
    y2 += gvb
    y2 += np.asarray(x2, dtype=np.float32)
    y1 += gvb
    y1 += np.asarray(x1, dtype=np.float32)
    return y2, y1


def unpermute(yt):
    # yt [C, s''=j2*128+wp*64+h] -> y[c, h, w=2*j2+wp]
    return np.ascontiguousarray(
        yt.astype(np.float32).reshape(C, 32, 2, 64).transpose(0, 3, 1, 2).reshape(C, H, W)
    )


def kernel(x2, x1, q_w, q_b, k_w, k_b, v_w, v_b, gamma):
    in_maps, g = make_in_maps(x2, x1, q_w, q_b, k_w, k_b, v_w, v_b, gamma)
    if "nc" not in _CACHED:
        _CACHED["nc"] = build_nc()
    nc = _CACHED["nc"]
    res = run_bass_kernel_spmd(nc, in_maps, list(range(B))).results
    return assemble_outputs(res, x2, x1, v_b, g)
